# revision 4
# baseline (speedup 1.0000x reference)
"""CrossAttention2D (16-head, 2D-RoPE) Trainium2 kernel.

Sharding: 8 cores = (batch b = c//2) x (query-token half t = c%2).
Each core projects Q for its 1152 tokens, K/V for the full 2304 tokens of
its batch (K/V projection replicated between the 2 cores of a batch, which
avoids all collectives), runs all 16 heads of attention for its query
tokens, and produces a disjoint [1152, 1024] slice of the final output
(transposed); the host concatenates.

All matmuls run in float32r (full-rate on the PE at fp32-level accuracy).
Softmax is computed without max-subtraction (scores are ~N(0,1), exp is
safe in fp32); the denominator comes from an all-ones 65th column appended
to V so the same PE pass that computes attn@V also produces sum(exp).
"""

import os
import numpy as np

B, N, D = 4, 2304, 1024
NH, HD = 16, 64
NQ = N // 2          # query tokens per core
ICH = 384            # i-chunk (attention free dim per matmul)
NIC = NQ // ICH      # 3
NJC = N // 128       # 18 key blocks
NDC = D // 128       # 8 contraction blocks

_STATE = {}
LAST_EXEC_NS = None
LAST_RESULTS = None


def _build():
    import concourse.tile as tile
    from concourse import bacc, mybir
    from contextlib import ExitStack

    f32 = mybir.dt.float32
    f32r = mybir.dt.float32r
    AF = mybir.ActivationFunctionType

    nc = bacc.Bacc("TRN2", target_bir_lowering=False, debug=False, num_devices=8)

    def din(name, shape):
        return nc.dram_tensor(name, shape, f32, kind="ExternalInput").ap()

    qT = din("qT", [D, NQ])
    kT = din("kT", [D, N])
    vT = din("vT", [D, N])
    w_in = {"wq": din("wq", [D, D]), "wk": din("wk", [D, D]),
            "wv": din("wv", [D, D]), "wo": din("wo", [D, D])}
    b_in = {"bq": din("bq", [D]), "bk": din("bk", [D]),
            "bv": din("bv", [D]), "bo": din("bo", [D])}
    cos_q_d = din("cos_q", [128, NQ])
    sin_q_d = din("sin_q", [128, NQ])
    cos_k_d = din("cos_k", [128, N])
    sin_k_d = din("sin_k", [128, N])
    swp_d = din("swp", [128, 128])
    outT = nc.dram_tensor("outT", [D, NQ], f32, kind="ExternalOutput").ap()

    with tile.TileContext(nc) as tc:
        with ExitStack() as top:
            dram = top.enter_context(tc.tile_pool(name="dram", bufs=1, space="DRAM"))
            qr = dram.tile([D, NQ], f32r, tag="qr")
            kr = dram.tile([D, N], f32r, tag="kr")
            vp = dram.tile([N, D], f32r, tag="vp")

            const = top.enter_context(tc.tile_pool(name="const", bufs=1))
            bias_sb = {}
            for nm in ("bq", "bk", "bo"):
                t = const.tile([128, NDC], f32, tag=nm)
                nc.sync.dma_start(out=t, in_=b_in[nm].rearrange("(c p) -> p c", p=128))
                bias_sb[nm] = t
            bv_sb = const.tile([1, D], f32r, tag="bv")
            nc.sync.dma_start(out=bv_sb, in_=b_in["bv"][None, :].bitcast(f32r))
            swp_sb = const.tile([128, 128], f32r, tag="swp")
            nc.sync.dma_start(out=swp_sb, in_=swp_d.bitcast(f32r))
            ones1f = const.tile([1, 128], f32, tag="ones1f")
            nc.vector.memset(ones1f, 1.0)
            ones1 = const.tile([1, 128], f32r, tag="ones1")
            nc.vector.tensor_copy(out=ones1, in_=ones1f)
            ones2f = const.tile([128, 2, 1], f32, tag="ones2f")
            nc.vector.memset(ones2f, 1.0)
            cs = {}
            for nm, src, nn in (("cq", cos_q_d, NQ), ("sq", sin_q_d, NQ),
                                ("ck", cos_k_d, N), ("sk", sin_k_d, N)):
                t = const.tile([128, nn], f32, tag=nm)
                nc.sync.dma_start(out=t, in_=src)
                cs[nm] = t

            # ---------------- Phase P: projections ----------------
            with ExitStack() as ph:
                wpool = ph.enter_context(tc.tile_pool(name="wqkv", bufs=1))
                xin = ph.enter_context(tc.tile_pool(name="xin", bufs=2))
                pp = ph.enter_context(tc.tile_pool(name="pp", bufs=2, space="PSUM"))
                psw = ph.enter_context(tc.tile_pool(name="psw", bufs=2, space="PSUM"))
                ptmp = ph.enter_context(tc.tile_pool(name="ptmp", bufs=3))

                w_sb = {}
                for nm in ("wq", "wk", "wv"):
                    for dc in range(NDC):
                        t = wpool.tile([128, D], f32r, tag=f"{nm}{dc}")
                        nc.sync.dma_start(
                            out=t, in_=w_in[nm][dc * 128:(dc + 1) * 128, :].bitcast(f32r))
                        w_sb[nm, dc] = t

                # Q and K projections -> transposed [o, n] layout + RoPE
                for wn, bn, src, dst, nn, ct, st in (
                        ("wk", "bk", kT, kr, N, cs["ck"], cs["sk"]),
                        ("wq", "bq", qT, qr, NQ, cs["cq"], cs["sq"])):
                    for ch in range(nn // ICH):
                        sl = slice(ch * ICH, (ch + 1) * ICH)
                        xs = []
                        for dc in range(NDC):
                            xt = xin.tile([128, ICH], f32r, tag=f"x{dc}")
                            nc.sync.dma_start(
                                out=xt, in_=src[dc * 128:(dc + 1) * 128, sl].bitcast(f32r))
                            xs.append(xt)
                        for ob in range(NDC):
                            ps = pp.tile([128, ICH], f32, tag="ps")
                            for dc in range(NDC):
                                nc.tensor.matmul(
                                    ps, w_sb[wn, dc][:, ob * 128:(ob + 1) * 128], xs[dc],
                                    start=(dc == 0), stop=(dc == NDC - 1))
                            xb = ptmp.tile([128, ICH], f32r, tag="xb")
                            nc.scalar.activation(out=xb, in_=ps, func=AF.Identity,
                                                 bias=bias_sb[bn][:, ob:ob + 1], scale=1.0)
                            sw = psw.tile([128, ICH], f32, tag="sw")
                            nc.tensor.matmul(sw, swp_sb, xb, start=True, stop=True)
                            t1 = ptmp.tile([128, ICH], f32, tag="t1")
                            nc.vector.tensor_mul(out=t1, in0=xb.bitcast(f32), in1=ct[:, sl])
                            t2 = ptmp.tile([128, ICH], f32, tag="t2")
                            nc.vector.tensor_mul(out=t2, in0=sw, in1=st[:, sl])
                            ro = ptmp.tile([128, ICH], f32r, tag="ro")
                            nc.vector.tensor_add(out=ro, in0=t1, in1=t2)
                            nc.sync.dma_start(
                                out=dst[ob * 128:(ob + 1) * 128, sl], in_=ro)

                # V projection -> natural [n, o] layout (+bias via ones-row matmul)
                for nb in range(NJC):
                    nsl = slice(nb * 128, (nb + 1) * 128)
                    vts = []
                    for dc in range(NDC):
                        vt = xin.tile([128, 128], f32r, tag=f"v{dc}")
                        nc.sync.dma_start(
                            out=vt, in_=vT[dc * 128:(dc + 1) * 128, nsl].bitcast(f32r))
                        vts.append(vt)
                    for oc in range(2):
                        osl = slice(oc * 512, (oc + 1) * 512)
                        ps = pp.tile([128, 512], f32, tag="pv")
                        for dc in range(NDC):
                            nc.tensor.matmul(ps, vts[dc], w_sb["wv", dc][:, osl],
                                             start=(dc == 0), stop=False)
                        nc.tensor.matmul(ps, ones1, bv_sb[:, osl], start=False, stop=True)
                        vo = ptmp.tile([128, 512], f32r, tag="vo")
                        nc.scalar.activation(out=vo, in_=ps, func=AF.Copy)
                        nc.sync.dma_start(out=vp[nsl, osl], in_=vo)

            # ---------------- Phase A: attention ----------------
            ctxp = top.enter_context(tc.tile_pool(name="ctxp", bufs=8))
            ctx_tiles = []
            with ExitStack() as ph:
                qkp = ph.enter_context(tc.tile_pool(name="qk", bufs=2))
                vaugp = ph.enter_context(tc.tile_pool(name="vaug", bufs=2))
                psp = ph.enter_context(tc.tile_pool(name="psp", bufs=2, space="PSUM"))
                pcp = ph.enter_context(tc.tile_pool(name="pcp", bufs=2, space="PSUM"))
                epp = ph.enter_context(tc.tile_pool(name="epp", bufs=3))
                rp = ph.enter_context(tc.tile_pool(name="rp", bufs=2))

                for hp in range(8):
                    hsl = slice(hp * 128, (hp + 1) * 128)
                    qp_t = qkp.tile([128, NQ], f32r, tag="qp")
                    nc.sync.dma_start(out=qp_t, in_=qr[hsl, :])
                    kp_t = qkp.tile([128, N], f32r, tag="kp")
                    nc.sync.dma_start(out=kp_t, in_=kr[hsl, :])
                    vas = []
                    for jc in range(NJC):
                        va = vaugp.tile([128, 2, 65], f32r, tag=f"va{jc}")
                        nc.sync.dma_start(
                            out=va[:, :, 0:64],
                            in_=vp[jc * 128:(jc + 1) * 128, hsl].rearrange(
                                "p (two c) -> p two c", two=2))
                        nc.vector.tensor_copy(out=va[:, :, 64:65], in_=ones2f)
                        vas.append(va)
                    ctx_t = ctxp.tile([128, NQ], f32r, tag="ctx")
                    ctx_tiles.append(ctx_t)
                    for ic in range(NIC):
                        isl = slice(ic * ICH, (ic + 1) * ICH)
                        pc0 = pcp.tile([65, ICH], f32, tag="pc0")
                        pc1 = pcp.tile([65, ICH], f32, tag="pc1")
                        for jc in range(NJC):
                            jsl = slice(jc * 128, (jc + 1) * 128)
                            ps0 = psp.tile([128, ICH], f32, tag="ps0")
                            ps1 = psp.tile([128, ICH], f32, tag="ps1")
                            nc.tensor.matmul(ps0, kp_t[0:64, jsl], qp_t[0:64, isl],
                                             start=True, stop=True)
                            nc.tensor.matmul(ps1, kp_t[64:128, jsl], qp_t[64:128, isl],
                                             start=True, stop=True)
                            e0 = epp.tile([128, ICH], f32r, tag="e0")
                            e1 = epp.tile([128, ICH], f32r, tag="e1")
                            nc.scalar.activation(out=e0, in_=ps0, func=AF.Exp, scale=0.125)
                            nc.scalar.activation(out=e1, in_=ps1, func=AF.Exp, scale=0.125)
                            nc.tensor.matmul(pc0, vas[jc][:, 0, :], e0,
                                             start=(jc == 0), stop=(jc == NJC - 1))
                            nc.tensor.matmul(pc1, vas[jc][:, 1, :], e1,
                                             start=(jc == 0), stop=(jc == NJC - 1))
                        r0 = rp.tile([1, ICH], f32, tag="r0")
                        r1 = rp.tile([1, ICH], f32, tag="r1")
                        nc.vector.reciprocal(out=r0, in_=pc0[64:65, :])
                        nc.vector.reciprocal(out=r1, in_=pc1[64:65, :])
                        rb0 = rp.tile([64, ICH], f32, tag="rb0")
                        rb1 = rp.tile([64, ICH], f32, tag="rb1")
                        nc.gpsimd.partition_broadcast(rb0, r0)
                        nc.gpsimd.partition_broadcast(rb1, r1)
                        nc.vector.tensor_mul(out=ctx_t[0:64, isl], in0=pc0[0:64, :], in1=rb0)
                        nc.vector.tensor_mul(out=ctx_t[64:128, isl], in0=pc1[0:64, :], in1=rb1)

            # ---------------- Phase O: output projection ----------------
            with ExitStack() as ph:
                wop = ph.enter_context(tc.tile_pool(name="wop", bufs=1))
                po = ph.enter_context(tc.tile_pool(name="po", bufs=2, space="PSUM"))
                outp = ph.enter_context(tc.tile_pool(name="outp", bufs=3))
                wo_sb = []
                for dc in range(NDC):
                    t = wop.tile([128, D], f32r, tag=f"wo{dc}")
                    nc.sync.dma_start(
                        out=t, in_=w_in["wo"][dc * 128:(dc + 1) * 128, :].bitcast(f32r))
                    wo_sb.append(t)
                for ob in range(NDC):
                    for ic in range(NIC):
                        isl = slice(ic * ICH, (ic + 1) * ICH)
                        po_t = po.tile([128, ICH], f32, tag="po")
                        for dc in range(NDC):
                            nc.tensor.matmul(
                                po_t, wo_sb[dc][:, ob * 128:(ob + 1) * 128],
                                ctx_tiles[dc][:, isl],
                                start=(dc == 0), stop=(dc == NDC - 1))
                        ot = outp.tile([128, ICH], f32, tag="ot")
                        nc.scalar.activation(out=ot, in_=po_t, func=AF.Identity,
                                             bias=bias_sb["bo"][:, ob:ob + 1], scale=1.0)
                        nc.sync.dma_start(
                            out=outT[ob * 128:(ob + 1) * 128, isl], in_=ot)

    nc.compile()
    return nc


def _rope_tables(h_patch, w_patch):
    n = h_patch * w_patch
    yy, xx = np.meshgrid(np.arange(h_patch), np.arange(w_patch), indexing="ij")
    y = yy.reshape(-1).astype(np.float32)
    x = xx.reshape(-1).astype(np.float32)
    half = HD // 2
    inv = (1.0 / (10000.0 ** (np.arange(0, half, 2, dtype=np.float32) / half))).astype(np.float32)
    fh = y[:, None] * inv[None, :]
    fw = x[:, None] * inv[None, :]
    emb = np.concatenate([fh, fh, fw, fw], axis=1)  # [n, 64]
    cos = np.cos(emb).astype(np.float32)
    sin = np.sin(emb).astype(np.float32)
    sign = np.where((np.arange(HD) % 32) < 16, -1.0, 1.0).astype(np.float32)
    cos_t = np.ascontiguousarray(np.tile(cos.T, (2, 1)))          # [128, n]
    sin_t = np.ascontiguousarray(np.tile((sin * sign).T, (2, 1)))  # [128, n]
    return cos_t, sin_t


def kernel(query, key, value, Wq, bq, Wk, bk, Wv, bv, Wo, bo, H_patch, W_patch):
    global LAST_EXEC_NS, LAST_RESULTS
    from concourse import bass_utils

    query = np.ascontiguousarray(np.asarray(query, dtype=np.float32))
    key = np.ascontiguousarray(np.asarray(key, dtype=np.float32))
    value = np.ascontiguousarray(np.asarray(value, dtype=np.float32))
    h_patch, w_patch = int(H_patch), int(W_patch)

    if "nc" not in _STATE:
        _STATE["nc"] = _build()
    nc = _STATE["nc"]

    cos_t, sin_t = _rope_tables(h_patch, w_patch)
    p = np.arange(128)
    partner = np.where(p % 32 < 16, p + 16, p - 16)
    S = np.zeros((128, 128), np.float32)
    S[p, partner] = 1.0
    swp = np.ascontiguousarray(S.T)

    shared = {
        "wq": np.ascontiguousarray(np.asarray(Wq, np.float32).T),
        "wk": np.ascontiguousarray(np.asarray(Wk, np.float32).T),
        "wv": np.ascontiguousarray(np.asarray(Wv, np.float32).T),
        "wo": np.ascontiguousarray(np.asarray(Wo, np.float32).T),
        "bq": np.asarray(bq, np.float32), "bk": np.asarray(bk, np.float32),
        "bv": np.asarray(bv, np.float32), "bo": np.asarray(bo, np.float32),
        "cos_k": cos_t, "sin_k": sin_t, "swp": swp,
    }
    kT_b = [np.ascontiguousarray(key[b].T) for b in range(B)]
    vT_b = [np.ascontiguousarray(value[b].T) for b in range(B)]

    in_maps = []
    for c in range(8):
        b, t = c // 2, c % 2
        tsl = slice(t * NQ, (t + 1) * NQ)
        m = dict(shared)
        m["qT"] = np.ascontiguousarray(query[b, tsl, :].T)
        m["kT"] = kT_b[b]
        m["vT"] = vT_b[b]
        m["cos_q"] = np.ascontiguousarray(cos_t[:, tsl])
        m["sin_q"] = np.ascontiguousarray(sin_t[:, tsl])
        in_maps.append(m)

    trace = bool(os.environ.get("KERNEL_TRACE"))
    kw = {}
    if trace and os.environ.get("KERNEL_TRACE_DIR"):
        os.makedirs(os.environ["KERNEL_TRACE_DIR"], exist_ok=True)
        kw["tmpdir"] = os.environ["KERNEL_TRACE_DIR"]
    res = bass_utils.run_bass_kernel_spmd(
        nc, in_maps, core_ids=list(range(8)), trace=trace, **kw)
    LAST_EXEC_NS = res.exec_time_ns
    LAST_RESULTS = res

    out = np.empty((B, N, D), dtype=np.float32)
    for c in range(8):
        b, t = c // 2, c % 2
        out[b, t * NQ:(t + 1) * NQ, :] = res.results[c]["outT"].T
    return out


# revision 6
# speedup vs baseline: 1.7704x; 1.7704x over previous
"""CrossAttention2D (16-head, 2D-RoPE) Trainium2 kernel.

Sharding: 8 cores = (batch b = c//2) x (query-token half t = c%2).
Each core projects Q for its 1152 tokens, K/V for the full 2304 tokens of
its batch (K/V projection replicated between the 2 cores of a batch, which
avoids all collectives), runs all 16 heads of attention for its query
tokens, and produces a disjoint [1152, 1024] slice of the final output
(transposed); the host concatenates.

Matmul operands are bf16 (PE full rate; fp32 streams are SBUF-BW bound),
accumulation is fp32 in PSUM, softmax runs in fp32 on the scalar engine.
Softmax needs no max-subtraction (scores ~N(0,1)); the denominator comes
from an all-ones 65th column appended to V so the AV pass also produces
sum(exp).
"""

import os
import numpy as np

B, N, D = 4, 2304, 1024
NH, HD = 16, 64
NQ = N // 2          # query tokens per core
ICH = 384            # i-chunk (attention free dim per matmul)
NIC = NQ // ICH      # 3
NJC = N // 128       # 18 key blocks
NDC = D // 128       # 8 contraction blocks

_STATE = {}
LAST_EXEC_NS = None
LAST_RESULTS = None


def _build():
    import concourse.tile as tile
    from concourse import bacc, mybir
    from contextlib import ExitStack

    f32 = mybir.dt.float32
    bf16 = mybir.dt.float16
    AF = mybir.ActivationFunctionType

    nc = bacc.Bacc("TRN2", target_bir_lowering=False, debug=False, num_devices=8)

    def din(name, shape, dt=bf16):
        return nc.dram_tensor(name, shape, dt, kind="ExternalInput").ap()

    qT = din("qT", [D, NQ])
    kT = din("kT", [D, N])
    vT = din("vT", [D, N])
    w_in = {"wq": din("wq", [D, D]), "wk": din("wk", [D, D]),
            "wv": din("wv", [D, D]), "wo": din("wo", [D, D])}
    b_in = {"bq": din("bq", [D], f32), "bk": din("bk", [D], f32),
            "bv": din("bv", [D]), "bo": din("bo", [D], f32)}
    cos_q_d = din("cos_q", [128, NQ], f32)
    sin_q_d = din("sin_q", [128, NQ], f32)
    cos_k_d = din("cos_k", [128, N], f32)
    sin_k_d = din("sin_k", [128, N], f32)
    swp_d = din("swp", [128, 128])
    outT = nc.dram_tensor("outT", [D, NQ], f32, kind="ExternalOutput").ap()

    with tile.TileContext(nc) as tc:
        with ExitStack() as top:
            dram = top.enter_context(tc.tile_pool(name="dram", bufs=1, space="DRAM"))
            qr = dram.tile([D, NQ], bf16, tag="qr")
            kr = dram.tile([D, N], bf16, tag="kr")
            vp = dram.tile([N, D], bf16, tag="vp")

            const = top.enter_context(tc.tile_pool(name="const", bufs=1))
            bias_sb = {}
            for nm in ("bq", "bk", "bo"):
                t = const.tile([128, NDC], f32, tag=nm)
                nc.sync.dma_start(out=t, in_=b_in[nm].rearrange("(c p) -> p c", p=128))
                bias_sb[nm] = t
            bv_sb = const.tile([1, D], bf16, tag="bv")
            nc.sync.dma_start(out=bv_sb, in_=b_in["bv"][None, :])
            swp_sb = const.tile([128, 128], bf16, tag="swp")
            nc.sync.dma_start(out=swp_sb, in_=swp_d)
            ones1f = const.tile([1, 128], f32, tag="ones1f")
            nc.vector.memset(ones1f, 1.0)
            ones1 = const.tile([1, 128], bf16, tag="ones1")
            nc.vector.tensor_copy(out=ones1, in_=ones1f)
            ones2f = const.tile([128, 2, 1], f32, tag="ones2f")
            nc.vector.memset(ones2f, 1.0)
            cs = {}
            for nm, src, nn in (("cq", cos_q_d, NQ), ("sq", sin_q_d, NQ),
                                ("ck", cos_k_d, N), ("sk", sin_k_d, N)):
                t = const.tile([128, nn], f32, tag=nm)
                nc.sync.dma_start(out=t, in_=src)
                cs[nm] = t

            # ---------------- Phase P: projections ----------------
            with ExitStack() as ph:
                wpool = ph.enter_context(tc.tile_pool(name="wqkv", bufs=1))
                xin = ph.enter_context(tc.tile_pool(name="xin", bufs=2))
                pp = ph.enter_context(tc.tile_pool(name="pp", bufs=2, space="PSUM"))
                psw = ph.enter_context(tc.tile_pool(name="psw", bufs=2, space="PSUM"))
                ptmp = ph.enter_context(tc.tile_pool(name="ptmp", bufs=3))

                w_sb = {}
                for nm in ("wq", "wk", "wv"):
                    for dc in range(NDC):
                        t = wpool.tile([128, D], bf16, tag=f"{nm}{dc}")
                        nc.sync.dma_start(out=t, in_=w_in[nm][dc * 128:(dc + 1) * 128, :])
                        w_sb[nm, dc] = t

                # Q and K projections -> transposed [o, n] layout + RoPE
                for wn, bn, src, dst, nn, ct, st in (
                        ("wk", "bk", kT, kr, N, cs["ck"], cs["sk"]),
                        ("wq", "bq", qT, qr, NQ, cs["cq"], cs["sq"])):
                    for ch in range(nn // ICH):
                        sl = slice(ch * ICH, (ch + 1) * ICH)
                        xs = []
                        for dc in range(NDC):
                            xt = xin.tile([128, ICH], bf16, tag=f"x{dc}")
                            nc.sync.dma_start(out=xt, in_=src[dc * 128:(dc + 1) * 128, sl])
                            xs.append(xt)
                        for ob in range(NDC):
                            ps = pp.tile([128, ICH], f32, tag="ps")
                            for dc in range(NDC):
                                nc.tensor.matmul(
                                    ps, w_sb[wn, dc][:, ob * 128:(ob + 1) * 128], xs[dc],
                                    start=(dc == 0), stop=(dc == NDC - 1))
                            xb = ptmp.tile([128, ICH], bf16, tag="xb")
                            nc.vector.tensor_scalar_add(
                                out=xb, in0=ps, scalar1=bias_sb[bn][:, ob:ob + 1])
                            sw = psw.tile([128, ICH], f32, tag="sw")
                            nc.tensor.matmul(sw, swp_sb, xb, start=True, stop=True)
                            t1 = ptmp.tile([128, ICH], f32, tag="t1")
                            nc.vector.tensor_mul(out=t1, in0=xb, in1=ct[:, sl])
                            t2 = ptmp.tile([128, ICH], f32, tag="t2")
                            nc.vector.tensor_mul(out=t2, in0=sw, in1=st[:, sl])
                            ro = ptmp.tile([128, ICH], bf16, tag="ro")
                            nc.vector.tensor_add(out=ro, in0=t1, in1=t2)
                            nc.sync.dma_start(
                                out=dst[ob * 128:(ob + 1) * 128, sl], in_=ro)

                # V projection -> natural [n, o] layout (+bias via ones-row matmul)
                for nb in range(NJC):
                    nsl = slice(nb * 128, (nb + 1) * 128)
                    vts = []
                    for dc in range(NDC):
                        vt = xin.tile([128, 128], bf16, tag=f"v{dc}")
                        nc.sync.dma_start(out=vt, in_=vT[dc * 128:(dc + 1) * 128, nsl])
                        vts.append(vt)
                    for oc in range(2):
                        osl = slice(oc * 512, (oc + 1) * 512)
                        ps = pp.tile([128, 512], f32, tag="pv")
                        for dc in range(NDC):
                            nc.tensor.matmul(ps, vts[dc], w_sb["wv", dc][:, osl],
                                             start=(dc == 0), stop=False)
                        nc.tensor.matmul(ps, ones1, bv_sb[:, osl], start=False, stop=True)
                        vo = ptmp.tile([128, 512], bf16, tag="vo")
                        nc.vector.tensor_copy(out=vo, in_=ps)
                        nc.sync.dma_start(out=vp[nsl, osl], in_=vo)

            # ---------------- Phase A: attention ----------------
            ctxp = top.enter_context(tc.tile_pool(name="ctxp", bufs=8))
            ctx_tiles = []
            with ExitStack() as ph:
                qkp = ph.enter_context(tc.tile_pool(name="qk", bufs=2))
                vaugp = ph.enter_context(tc.tile_pool(name="vaug", bufs=2))
                psp = ph.enter_context(tc.tile_pool(name="psp", bufs=2, space="PSUM"))
                pcp = ph.enter_context(tc.tile_pool(name="pcp", bufs=2, space="PSUM"))
                epp = ph.enter_context(tc.tile_pool(name="epp", bufs=3))
                rp = ph.enter_context(tc.tile_pool(name="rp", bufs=2))

                for hp in range(8):
                    hsl = slice(hp * 128, (hp + 1) * 128)
                    qp_t = qkp.tile([128, NQ], bf16, tag="qp")
                    nc.sync.dma_start(out=qp_t, in_=qr[hsl, :])
                    kp_t = qkp.tile([128, N], bf16, tag="kp")
                    nc.sync.dma_start(out=kp_t, in_=kr[hsl, :])
                    vas = []
                    for jc in range(NJC):
                        va = vaugp.tile([128, 2, 65], bf16, tag=f"va{jc}")
                        nc.sync.dma_start(
                            out=va[:, :, 0:64],
                            in_=vp[jc * 128:(jc + 1) * 128, hsl].rearrange(
                                "p (two c) -> p two c", two=2))
                        nc.vector.tensor_copy(out=va[:, :, 64:65], in_=ones2f)
                        vas.append(va)
                    ctx_t = ctxp.tile([128, NQ], bf16, tag="ctx")
                    ctx_tiles.append(ctx_t)
                    for ic in range(NIC):
                        isl = slice(ic * ICH, (ic + 1) * ICH)
                        pc0 = pcp.tile([65, ICH], f32, tag="pc0")
                        pc1 = pcp.tile([65, ICH], f32, tag="pc1")
                        for jc in range(NJC):
                            jsl = slice(jc * 128, (jc + 1) * 128)
                            # two head-packed score matmuls into one 2-bank tile
                            pf = psp.tile([128, 2, 512], f32, tag="pf")
                            nc.tensor.matmul(pf[:, 0, 0:ICH], kp_t[0:64, jsl],
                                             qp_t[0:64, isl], start=True, stop=True)
                            nc.tensor.matmul(pf[:, 1, 0:ICH], kp_t[64:128, jsl],
                                             qp_t[64:128, isl], start=True, stop=True)
                            e01 = epp.tile([128, 2, ICH], bf16, tag="e01")
                            nc.scalar.activation(out=e01, in_=pf[:, :, 0:ICH],
                                                 func=AF.Exp, scale=0.125)
                            nc.tensor.matmul(pc0, vas[jc][:, 0, :], e01[:, 0, :],
                                             start=(jc == 0), stop=(jc == NJC - 1))
                            nc.tensor.matmul(pc1, vas[jc][:, 1, :], e01[:, 1, :],
                                             start=(jc == 0), stop=(jc == NJC - 1))
                        r0 = rp.tile([1, ICH], f32, tag="r0")
                        r1 = rp.tile([1, ICH], f32, tag="r1")
                        nc.vector.reciprocal(out=r0, in_=pc0[64:65, :])
                        nc.vector.reciprocal(out=r1, in_=pc1[64:65, :])
                        rb0 = rp.tile([64, ICH], f32, tag="rb0")
                        rb1 = rp.tile([64, ICH], f32, tag="rb1")
                        nc.gpsimd.partition_broadcast(rb0, r0)
                        nc.gpsimd.partition_broadcast(rb1, r1)
                        nc.vector.tensor_mul(out=ctx_t[0:64, isl], in0=pc0[0:64, :], in1=rb0)
                        nc.vector.tensor_mul(out=ctx_t[64:128, isl], in0=pc1[0:64, :], in1=rb1)

            # ---------------- Phase O: output projection ----------------
            with ExitStack() as ph:
                wop = ph.enter_context(tc.tile_pool(name="wop", bufs=1))
                po = ph.enter_context(tc.tile_pool(name="po", bufs=2, space="PSUM"))
                outp = ph.enter_context(tc.tile_pool(name="outp", bufs=3))
                wo_sb = []
                for dc in range(NDC):
                    t = wop.tile([128, D], bf16, tag=f"wo{dc}")
                    nc.sync.dma_start(out=t, in_=w_in["wo"][dc * 128:(dc + 1) * 128, :])
                    wo_sb.append(t)
                for ob in range(NDC):
                    for ic in range(NIC):
                        isl = slice(ic * ICH, (ic + 1) * ICH)
                        po_t = po.tile([128, ICH], f32, tag="po")
                        for dc in range(NDC):
                            nc.tensor.matmul(
                                po_t, wo_sb[dc][:, ob * 128:(ob + 1) * 128],
                                ctx_tiles[dc][:, isl],
                                start=(dc == 0), stop=(dc == NDC - 1))
                        ot = outp.tile([128, ICH], f32, tag="ot")
                        nc.scalar.activation(out=ot, in_=po_t, func=AF.Identity,
                                             bias=bias_sb["bo"][:, ob:ob + 1], scale=1.0)
                        nc.sync.dma_start(
                            out=outT[ob * 128:(ob + 1) * 128, isl], in_=ot)

    nc.compile()
    return nc


def _rope_tables(h_patch, w_patch):
    yy, xx = np.meshgrid(np.arange(h_patch), np.arange(w_patch), indexing="ij")
    y = yy.reshape(-1).astype(np.float32)
    x = xx.reshape(-1).astype(np.float32)
    half = HD // 2
    inv = (1.0 / (10000.0 ** (np.arange(0, half, 2, dtype=np.float32) / half))).astype(np.float32)
    fh = y[:, None] * inv[None, :]
    fw = x[:, None] * inv[None, :]
    emb = np.concatenate([fh, fh, fw, fw], axis=1)  # [n, 64]
    cos = np.cos(emb).astype(np.float32)
    sin = np.sin(emb).astype(np.float32)
    sign = np.where((np.arange(HD) % 32) < 16, -1.0, 1.0).astype(np.float32)
    cos_t = np.ascontiguousarray(np.tile(cos.T, (2, 1)))          # [128, n]
    sin_t = np.ascontiguousarray(np.tile((sin * sign).T, (2, 1)))  # [128, n]
    return cos_t, sin_t


def kernel(query, key, value, Wq, bq, Wk, bk, Wv, bv, Wo, bo, H_patch, W_patch):
    global LAST_EXEC_NS, LAST_RESULTS
    import ml_dtypes
    from concourse import bass_utils

    bf = np.float16
    query = np.asarray(query, dtype=np.float32)
    key = np.asarray(key, dtype=np.float32)
    value = np.asarray(value, dtype=np.float32)
    h_patch, w_patch = int(H_patch), int(W_patch)

    if "nc" not in _STATE:
        _STATE["nc"] = _build()
    nc = _STATE["nc"]

    cos_t, sin_t = _rope_tables(h_patch, w_patch)
    p = np.arange(128)
    partner = np.where(p % 32 < 16, p + 16, p - 16)
    S = np.zeros((128, 128), np.float32)
    S[p, partner] = 1.0

    shared = {
        "wq": np.asarray(Wq, np.float32).T.astype(bf),
        "wk": np.asarray(Wk, np.float32).T.astype(bf),
        "wv": np.asarray(Wv, np.float32).T.astype(bf),
        "wo": np.asarray(Wo, np.float32).T.astype(bf),
        "bq": np.asarray(bq, np.float32), "bk": np.asarray(bk, np.float32),
        "bv": np.asarray(bv, np.float32).astype(bf), "bo": np.asarray(bo, np.float32),
        "cos_k": cos_t, "sin_k": sin_t, "swp": S.T.astype(bf),
    }
    kT_b = [key[b].T.astype(bf) for b in range(B)]
    vT_b = [value[b].T.astype(bf) for b in range(B)]

    in_maps = []
    for c in range(8):
        b, t = c // 2, c % 2
        tsl = slice(t * NQ, (t + 1) * NQ)
        m = dict(shared)
        m["qT"] = query[b, tsl, :].T.astype(bf)
        m["kT"] = kT_b[b]
        m["vT"] = vT_b[b]
        m["cos_q"] = np.ascontiguousarray(cos_t[:, tsl])
        m["sin_q"] = np.ascontiguousarray(sin_t[:, tsl])
        in_maps.append(m)

    trace = bool(os.environ.get("KERNEL_TRACE"))
    kw = {}
    if trace and os.environ.get("KERNEL_TRACE_DIR"):
        os.makedirs(os.environ["KERNEL_TRACE_DIR"], exist_ok=True)
        kw["tmpdir"] = os.environ["KERNEL_TRACE_DIR"]
    res = bass_utils.run_bass_kernel_spmd(
        nc, in_maps, core_ids=list(range(8)), trace=trace, **kw)
    LAST_EXEC_NS = res.exec_time_ns
    LAST_RESULTS = res

    out = np.empty((B, N, D), dtype=np.float32)
    for c in range(8):
        b, t = c // 2, c % 2
        out[b, t * NQ:(t + 1) * NQ, :] = res.results[c]["outT"].T
    return out
